# revision 25
# baseline (speedup 1.0000x reference)
"""Multi-head self-attention on 8 Trainium2 NeuronCores.

Problem: x[4, 2048, 1024], 16 heads x 64 dims, fused qkv + attention + out-proj.

Sharding (hybrid, per the tensor-parallel hint): core c handles batch b = c//2
and head-group g = c%2 (8 of the 16 heads). Each core computes a partial
out-projection over its 8 heads; the host sums the two group partials per
batch and adds b_out.

Per-core kernel (all matmuls bf16, fp32 PSUM accumulation):
  - exp is SPLIT between the scalar engine (true exp out of PSUM) and the
    vector engine (Schraudolph bit-trick: i16 = trunc(psc + 16249) bitcast
    to bf16 ~= exp(s/8); the multiply 23.083*s is folded into the HOST-side
    q-projection weights so the DVE op is a single tensor_scalar_add - the
    fused two-op mult+add with int16 output drops products on real HW).
    7 of 16 k-chunks go to DVE; this halves the ACT time per unit, which
    was the baseline's critical path (ACT ~19us/unit vs PE ~13.7us/unit).
  - deep software pipeline: scores/exp of unit n+3 run during the AV of
    unit n (E tiles triple-buffered; the E ring shares its 32KB slots with
    xT, which dies once the prologue projections finish). The psum scores
    tile recycle (bufs=2) then references exp work from a full iteration
    ago, so the PE never stalls on ACT/DVE at unit boundaries and the HAM
    clock stays warm.
  - normalize is split: one ACT copy retires the pw PSUM slot into SBUF at
    the end of each iteration (plus ln + exp(-x) reciprocal on the pinned
    exp/ln table), then next iteration a GPSIMD partition_broadcast fans
    the reciprocal across 64 partitions and one DVE 2x-mode multiply writes
    the normalized waT. No PE involvement at all.
  - scores computed transposed (S^T[k, q] = kT.T @ qT) per 128-row k-chunk,
    with the two heads of a pair row-packed on disjoint PE row groups.
  - softmax denominator comes free as an all-ones column appended to V in
    the AV matmul.
  - the first 6 k-proj tiles run contraction-outer across idle PSUM banks
    so the PE tracks the arriving x DMA instead of idling per chunk.
  - qk-proj bias is applied by the ACT engine (Identity+bias PSUM->SBUF
    move) during the projection prologue where ACT is otherwise idle; the
    v-bias is folded EXACTLY into the host-side output constant
    (wa @ (v + bv) = wa @ v + bv since attention weights sum to 1).
  - out-proj staged in bf16 (halves the output DMA); host sums the two
    head-group partials in fp32. The final q-range's out-proj partially
    accumulates during the last AV chain to shorten the tail.
"""

import os
import sys
from contextlib import ExitStack

import numpy as np

for _p in ("/opt/trn_rl_repo",):
    if _p not in sys.path and os.path.isdir(_p):
        sys.path.insert(0, _p)

import ml_dtypes

import concourse.bass as bass
import concourse.tile as tile
from concourse import bacc, mybir
from concourse.bass_utils import run_bass_kernel_spmd

BF16 = ml_dtypes.bfloat16
F32 = np.float32

D = 1024
H = 16
HD = 64
B = 4
N = 2048
NCORES = 8
G = 2  # head groups (tensor-parallel axis)
LH = H // G  # local heads per core
DC = D // 128  # 8 contraction chunks
KC = N // 128  # 16 k-token chunks
QT = N // 512  # 4 q tiles
TOK = N // 128  # 16 token chunks

# k-chunks whose exp runs on DVE (Schraudolph bit-trick) instead of ACT
DVE_SET = frozenset((1, 3, 5, 7, 9, 11, 13, 15))
QSCALE = float(0.125 * 128.0 / np.log(2.0))  # folded into host q weights
EXP_B = 16249.0  # bf16 exponent bias + rms-optimal shift (trunc-calibrated)
ACT_SCALE = float(0.125 / QSCALE)  # undo the fold for the true-exp path

_CACHE = {}


def _pin_act_tables():
    """Make the act-table chooser resolve exp AND ln to the one set that
    holds both (natural_log_exp_and_others), instead of thrashing between
    exp_and_others and natural_log on every softmax/reciprocal boundary
    (~1.3us ACT stall per reload). Other sets keep their index/id; we only
    hide exp/ln from them so they are never chosen for those funcs.
    """
    if _CACHE.get("act_pinned"):
        return
    from concourse import bacc as _bacc
    from concourse import hw_specs as _hw

    orig = _hw.get_activation_tables

    def patched(arch):
        t = dict(orig(arch))
        keep = "natural_log_exp_and_others"
        if keep in t:
            pinned = t[keep]
            t = {n: (s if n == keep else (s - pinned)) for n, s in t.items()}
        return t

    _hw.get_activation_tables = patched
    _bacc.get_activation_tables = patched
    _CACHE["act_pinned"] = True


def _build_nc():
    _pin_act_tables()
    nc = bacc.Bacc(None, target_bir_lowering=False)

    xT = nc.declare_dram_parameter("xT", [128, DC, N], mybir.dt.bfloat16, isOutput=False)
    # wqk[:, kc, 0, :] = k-features (4 pairs x 128), [:, kc, 1, :] = q-features
    wqk = nc.declare_dram_parameter("wqk", [128, DC, 2, 512], mybir.dt.bfloat16, isOutput=False)
    bqk = nc.declare_dram_parameter("bqk", [128, 8], mybir.dt.float32, isOutput=False)
    wv = nc.declare_dram_parameter("wv", [128, DC, LH * HD], mybir.dt.bfloat16, isOutput=False)
    wout = nc.declare_dram_parameter("wout", [128, LH * HD // 128, D], mybir.dt.bfloat16, isOutput=False)
    out = nc.declare_dram_parameter("out", [N, D], mybir.dt.bfloat16, isOutput=True)

    with tile.TileContext(nc) as tc, ExitStack() as ctx:
        const = ctx.enter_context(tc.tile_pool(name="const", bufs=1))
        big = ctx.enter_context(tc.tile_pool(name="big", bufs=4))
        work = ctx.enter_context(tc.tile_pool(name="work", bufs=1))
        outp = ctx.enter_context(tc.tile_pool(name="outp", bufs=2))
        small = ctx.enter_context(tc.tile_pool(name="small", bufs=2))
        ps_s = ctx.enter_context(tc.tile_pool(name="ps_s", bufs=2, space="PSUM"))
        ps_wa = ctx.enter_context(tc.tile_pool(name="ps_wa", bufs=2, space="PSUM"))
        ps_m = ctx.enter_context(tc.tile_pool(name="ps_m", bufs=2, space="PSUM"))

        bqk_sb = const.tile([128, 8], mybir.dt.float32)
        # xT, wqk and wv share the 32KB big-pool ring with the E tiles:
        # they occupy slots during the projection prologue and are recycled
        # by E(1), E(2) and E(3) once their last readers retire.
        xT_sb = big.tile([128, DC, N], mybir.dt.bfloat16, tag="big", name="xT")
        wqk_sb = big.tile([128, DC, 2, 512], mybir.dt.bfloat16, tag="big", name="wqk")
        wv_sb = big.tile([128, DC, LH * HD], mybir.dt.bfloat16, tag="big", name="wv")
        nc.sync.dma_start(out=bqk_sb[:], in_=bqk[:])
        wout_sb = const.tile([128, LH * HD // 128, D], mybir.dt.bfloat16)
        # qkT[:, 0, p, :] = k-features of pair p; [:, 1, p, :] = q-features
        qkT_sb = work.tile([128, 2, 4, N], mybir.dt.bfloat16, tag="qkT")
        V_sb = work.tile([128, KC, LH, HD + 1], mybir.dt.bfloat16, tag="V")
        wa_pool = ctx.enter_context(tc.tile_pool(name="wa_pool", bufs=2))
        waT_ring = {}

        # ones column (index HD) for the free softmax denominator; the v-proj
        # copies below only fill [0:HD] so the column survives.
        nc.vector.memset(V_sb[:, :, :, HD : HD + 1], 1.0)

        def emit_proj(t, p, tt):
            """One [128 feats, 512 toks] tile of the q/k projection.
            t=0 -> k-features, t=1 -> q-features of pair p, token tile tt.
            PSUM->SBUF move with per-feature bias runs on ACT (idle here)."""
            pq = ps_m.tile([128, 512], mybir.dt.float32, tag="misc", name=f"pq_{t}_{p}_{tt}")
            for kc in range(DC):
                nc.tensor.matmul(
                    pq[:],
                    lhsT=wqk_sb[:, kc, t, p * 128 : (p + 1) * 128],
                    rhs=xT_sb[:, kc, tt * 512 : (tt + 1) * 512],
                    start=(kc == 0),
                    stop=(kc == DC - 1),
                )
            nc.scalar.activation(
                out=qkT_sb[:, t, p, tt * 512 : (tt + 1) * 512],
                in_=pq[:],
                func=mybir.ActivationFunctionType.Identity,
                bias=bqk_sb[:, t * 4 + p : t * 4 + p + 1],
            )

        def emit_vproj():
            # v projection: V[tok, feat] = x @ w_v. The v-bias is EXACT as a
            # host-side output constant (attention weights sum to 1, so
            # wa @ (v + bv) = wa @ v + bv): folded into b_out on the host.
            for c in range(TOK):
                pv = ps_m.tile([128, 512], mybir.dt.float32, tag="misc", name=f"pv_{c}")
                for kc in range(DC):
                    nc.tensor.matmul(
                        pv[:],
                        lhsT=xT_sb[:, kc, c * 128 : (c + 1) * 128],
                        rhs=wv_sb[:, kc, :],
                        start=(kc == 0),
                        stop=(kc == DC - 1),
                    )
                nc.vector.tensor_copy(
                    out=V_sb[:, c, :, 0:HD],
                    in_=pv[:].rearrange("p (l d) -> p l d", l=LH),
                )

        def unit(n):
            return n // 4, n % 4  # (q4, pair)

        def emit_scores_chunk(n, kc, E):
            q4, pair = unit(n)
            psc = ps_s.tile([128, 1024], mybir.dt.float32, tag="sc", name=f"sc_{n}_{kc}")
            for h01 in range(2):
                row = 64 * h01
                nc.tensor.matmul(
                    psc[:, h01 * 512 : (h01 + 1) * 512],
                    lhsT=qkT_sb[row : row + 64, 0, pair, kc * 128 : (kc + 1) * 128],
                    rhs=qkT_sb[row : row + 64, 1, pair, q4 * 512 : (q4 + 1) * 512],
                    start=True,
                    stop=True,
                )
            if kc in DVE_SET:
                # q-weights carry the 23.083 scale: one add + int16 convert
                # IS exp (Schraudolph), bitcast back to bf16 via the E view.
                nc.vector.tensor_scalar_add(
                    out=E[:, kc, :].bitcast(mybir.dt.int16),
                    in0=psc[:],
                    scalar1=EXP_B,
                )
            else:
                nc.scalar.activation(
                    out=E[:, kc, :],
                    in_=psc[:],
                    func=mybir.ActivationFunctionType.Exp,
                    scale=ACT_SCALE,
                )

        def emit_av_chunk(n, kc, E, pw, hs=(0, 1)):
            _, pair = unit(n)
            for h01 in hs:
                nc.tensor.matmul(
                    pw[h01][:],
                    lhsT=V_sb[:, kc, 2 * pair + h01, :],
                    rhs=E[:, kc, h01 * 512 : (h01 + 1) * 512],
                    start=(kc == 0),
                    stop=(kc == KC - 1),
                )

        def emit_norm_a(n, h01, pw):
            """Phase A (emitted at the end of unit n's iteration): one ACT
            copy retires the pw PSUM slot into SBUF so the next unit's AV
            chain can recycle it immediately, then ln + exp(-x) compute the
            reciprocal of the denominator row (same pinned act table as the
            softmax exp)."""
            pwS = small.tile([65, 512], mybir.dt.bfloat16, tag="pwS", name=f"pwS_{n}_{h01}")
            nc.scalar.activation(
                out=pwS[:], in_=pw[:], func=mybir.ActivationFunctionType.Copy
            )
            lg = small.tile([1, 512], mybir.dt.bfloat16, tag="lg", name=f"lg_{n}_{h01}")
            nc.scalar.activation(
                out=lg[:], in_=pwS[64:65, :], func=mybir.ActivationFunctionType.Ln
            )
            recip = small.tile([1, 512], mybir.dt.bfloat16, tag="recip", name=f"r_{n}_{h01}")
            nc.scalar.activation(
                out=recip[:],
                in_=lg[:],
                func=mybir.ActivationFunctionType.Exp,
                scale=-1.0,
            )
            return pwS, recip

        def emit_norm_b_pb(n, h01, recip):
            """Phase B part 1 (next iteration): broadcast the reciprocal
            across 64 partitions on the otherwise-idle GPSIMD engine (frees
            the PE of 32 rank-1 matmuls and their stationary-swap drains)."""
            rb = small.tile([64, 512], mybir.dt.bfloat16, tag="rb", name=f"rb_{n}_{h01}")
            nc.gpsimd.partition_broadcast(rb[:], recip[:])
            return rb

        def emit_norm_b_mul(n, h01, pwS, rb):
            """Phase B part 2 (DVE): normalized waT = wa * (1/denom); both
            operands SBUF bf16 so the DVE runs in its 2x mode."""
            q4, pair = unit(n)
            row = 64 * h01
            nc.vector.tensor_mul(
                out=waT_ring[q4][row : row + 64, pair, :],
                in0=pwS[0:64, :],
                in1=rb[:],
            )

        def emit_outproj(oq4):
            # out projection for a finished q-range; overlaps the next
            # q-range's attention stream. PSUM->SBUF moves alternate between
            # ACT and DVE to split the load; staged/stored in bf16.
            for cc in range(4):
                c = oq4 * 4 + cc
                o_sb = outp.tile([128, D], mybir.dt.bfloat16, tag="osb", name=f"o_{c}")
                for half in range(2):
                    po = ps_m.tile(
                        [128, 512], mybir.dt.float32, tag="misc", name=f"po_{c}_{half}"
                    )
                    for k4 in range(LH * HD // 128):
                        nc.tensor.matmul(
                            po[:],
                            lhsT=waT_ring[oq4][:, k4, cc * 128 : (cc + 1) * 128],
                            rhs=wout_sb[:, k4, half * 512 : (half + 1) * 512],
                            start=(k4 == 0),
                            stop=(k4 == LH * HD // 128 - 1),
                        )
                    dst = o_sb[:, half * 512 : (half + 1) * 512]
                    if half == 0:
                        nc.scalar.activation(
                            out=dst, in_=po[:], func=mybir.ActivationFunctionType.Copy
                        )
                    else:
                        nc.vector.tensor_copy(out=dst, in_=po[:])
                nc.sync.dma_start(out=out[c * 128 : (c + 1) * 128, :], in_=o_sb[:])

        # ---- prologue: all projections, then prime 3 units of scores ----
        # First 6 k-proj tiles run contraction-OUTER across idle PSUM banks
        # so each arriving xT chunk immediately feeds 6 matmuls and the PE
        # tracks the input DMA instead of idling ~1.4us per chunk.
        ko_sc = [
            ps_s.tile([128, 1024], mybir.dt.float32, tag="sc", name=f"ko_sc{i}")
            for i in range(2)
        ]
        ko_m = [
            ps_m.tile([128, 512], mybir.dt.float32, tag="misc", name=f"ko_m{i}")
            for i in range(2)
        ]
        ko_slots = [
            (ko_sc[0][:, 0:512], 0, 0), (ko_sc[0][:, 512:1024], 0, 1),
            (ko_sc[1][:, 0:512], 0, 2), (ko_sc[1][:, 512:1024], 0, 3),
            (ko_m[0][:], 1, 0), (ko_m[1][:], 1, 1),
        ]
        # DMA issues interleaved with the matmuls that consume them, so each
        # chunk's matmuls gate on just that chunk's two transfers instead of
        # a coarse all-inputs semaphore threshold.
        for kc in range(DC):
            nc.sync.dma_start(out=xT_sb[:, kc, :], in_=xT[:, kc, :])
            nc.sync.dma_start(out=wqk_sb[:, kc, 0, :], in_=wqk[:, kc, 0, :])
            for dst, p, tt in ko_slots:
                nc.tensor.matmul(
                    dst,
                    lhsT=wqk_sb[:, kc, 0, p * 128 : (p + 1) * 128],
                    rhs=xT_sb[:, kc, tt * 512 : (tt + 1) * 512],
                    start=(kc == 0),
                    stop=(kc == DC - 1),
                )
        for kc in range(DC):
            nc.sync.dma_start(out=wqk_sb[:, kc, 1, :], in_=wqk[:, kc, 1, :])
        nc.sync.dma_start(out=wv_sb[:], in_=wv[:])
        nc.sync.dma_start(out=wout_sb[:], in_=wout[:])
        for dst, p, tt in ko_slots:
            nc.scalar.activation(
                out=qkT_sb[:, 0, p, tt * 512 : (tt + 1) * 512],
                in_=dst,
                func=mybir.ActivationFunctionType.Identity,
                bias=bqk_sb[:, p : p + 1],
            )
        for p, tt in [(1, 2), (1, 3), (2, 0), (2, 1), (2, 2), (2, 3), (3, 0), (3, 1), (3, 2), (3, 3)]:
            emit_proj(0, p, tt)
        for p in range(4):
            emit_proj(1, p, 0)

        E_tiles = {}

        def alloc_E(n):
            E_tiles[n] = big.tile(
                [128, KC, 1024], mybir.dt.bfloat16, tag="big", name=f"E_{n}"
            )

        for n in (0, 1):
            alloc_E(n)
            for kc in range(KC):
                emit_scores_chunk(n, kc, E_tiles[n])
        for p in range(4):
            for tt in range(1, QT):
                emit_proj(1, p, tt)
        emit_vproj()  # last reader of xT; E(2) recycles its big-pool slot
        alloc_E(2)
        for kc in range(KC):
            emit_scores_chunk(2, kc, E_tiles[2])

        # ---- steady state: AV(n) interleaved with scores/exp(n+3) ----
        # Emission order per iteration is tuned so every cross-engine gate
        # (pw-slot recycle, psc-slot recycle, E-slot recycle) is satisfied
        # ~an iteration before the PE reaches the dependent instruction.
        LEAD = 3
        FRONT = 5
        norm_prev = None  # (pwS, recip) pairs of unit n-1
        pw_prev = None
        for n in range(15):
            m = n + LEAD
            if m <= 15:
                alloc_E(m)
            pw = [
                ps_wa.tile([65, 512], mybir.dt.float32, tag="wa", name=f"wa_{n}_{h}")
                for h in range(2)
            ]
            for kc in range(FRONT):
                emit_av_chunk(n, kc, E_tiles[n], pw)
            if m <= 15:
                for kc in range(FRONT):
                    emit_scores_chunk(m, kc, E_tiles[m])
            pbs = None
            if n >= 1:
                if (n - 1) % 4 == 0:
                    waT_ring[(n - 1) // 4] = wa_pool.tile(
                        [128, LH * HD // 128, 512], mybir.dt.bfloat16,
                        tag="waT", name=f"waT_{(n - 1) // 4}",
                    )
                pbs = [emit_norm_b_pb(n - 1, h, norm_prev[h][1]) for h in range(2)]
            for idx, j in enumerate(range(FRONT, KC - 1, 2)):
                emit_av_chunk(n, j, E_tiles[n], pw)
                emit_av_chunk(n, j + 1, E_tiles[n], pw)
                if m <= 15:
                    emit_scores_chunk(m, j, E_tiles[m])
                    emit_scores_chunk(m, j + 1, E_tiles[m])
                if idx == 1:
                    # muls late in the DVE queue (the exp chunks that gate
                    # the psc recycle come first); outproj right after the
                    # muls it depends on.
                    if pbs is not None:
                        for h01 in range(2):
                            emit_norm_b_mul(n - 1, h01, norm_prev[h01][0], pbs[h01])
                    if n % 4 == 0 and n > 0:
                        emit_outproj(n // 4 - 1)
            emit_av_chunk(n, KC - 1, E_tiles[n], pw)
            norm_prev = [emit_norm_a(n, h01, pw[h01]) for h01 in range(2)]
            if m <= 15:
                emit_scores_chunk(m, KC - 1, E_tiles[m])
            pw_prev = pw
            del E_tiles[n]
        # ---- unit 15: h0 chain -> its normalize overlaps the h1 chain, so
        # outproj(3) only waits on the short h1 normalize tail ----
        bcs14 = [emit_norm_b_pb(14, h, norm_prev[h][1]) for h in range(2)]
        pw = [
            ps_wa.tile([65, 512], mybir.dt.float32, tag="wa", name=f"wa_15_{h}")
            for h in range(2)
        ]
        waT_ring[3] = waT_ring.get(3) or wa_pool.tile(
            [128, LH * HD // 128, 512], mybir.dt.bfloat16, tag="waT", name="waT_3"
        )
        for kc in range(2):
            emit_av_chunk(15, kc, E_tiles[15], pw, hs=(0,))
        for h01 in range(2):
            emit_norm_b_mul(14, h01, norm_prev[h01][0], bcs14[h01])
        for kc in range(2, KC):
            emit_av_chunk(15, kc, E_tiles[15], pw, hs=(0,))
        n15_h0 = emit_norm_a(15, 0, pw[0])
        bc15_0 = emit_norm_b_pb(15, 0, n15_h0[1])
        emit_norm_b_mul(15, 0, n15_h0[0], bc15_0)
        for kc in range(KC):
            emit_av_chunk(15, kc, E_tiles[15], pw, hs=(1,))
        # last normalize skips the pwS staging (nothing recycles pw after
        # this) so outproj's final matmuls wait ~1us less.
        lg15 = small.tile([1, 512], mybir.dt.bfloat16, tag="lg", name="lg_15_1")
        nc.scalar.activation(
            out=lg15[:], in_=pw[1][64:65, :], func=mybir.ActivationFunctionType.Ln
        )
        r15 = small.tile([1, 512], mybir.dt.bfloat16, tag="recip", name="r_15_1")
        nc.scalar.activation(
            out=r15[:], in_=lg15[:], func=mybir.ActivationFunctionType.Exp, scale=-1.0
        )
        rb15 = small.tile([64, 512], mybir.dt.bfloat16, tag="rb", name="rb_15_1")
        nc.gpsimd.partition_broadcast(rb15[:], r15[:])
        # partial outproj chains for the first two token chunks of q4=3:
        # pairs 0-2 are normalized, so k4 0..2 accumulate during the tail
        pos_part = {}
        for cc in (0, 1):
            po = ps_m.tile([128, 512], mybir.dt.float32, tag="misc", name=f"po_p_{cc}")
            for k4 in range(3):
                nc.tensor.matmul(
                    po[:],
                    lhsT=waT_ring[3][:, k4, cc * 128 : (cc + 1) * 128],
                    rhs=wout_sb[:, k4, 0:512] if False else wout_sb[:, k4, :][:, 0:1024][:, 0:512],
                    start=(k4 == 0),
                    stop=False,
                )
            pos_part[cc] = po
        nc.vector.tensor_mul(
            out=waT_ring[3][64:128, 3, :], in0=pw[1][0:64, :], in1=rb15[:]
        )
        del E_tiles[15]
        # finish the partial chains (k4=3 needs unit 15's normalize), then
        # the remaining chunks
        for cc in (0, 1):
            c = 12 + cc
            o_sb = outp.tile([128, D], mybir.dt.bfloat16, tag="osb", name=f"o_{c}")
            po = pos_part[cc]
            nc.tensor.matmul(
                po[:],
                lhsT=waT_ring[3][:, 3, cc * 128 : (cc + 1) * 128],
                rhs=wout_sb[:, 3, 0:512],
                start=False,
                stop=True,
            )
            nc.scalar.activation(
                out=o_sb[:, 0:512], in_=po[:], func=mybir.ActivationFunctionType.Copy
            )
            po2 = ps_m.tile([128, 512], mybir.dt.float32, tag="misc", name=f"po_p2_{cc}")
            for k4 in range(4):
                nc.tensor.matmul(
                    po2[:],
                    lhsT=waT_ring[3][:, k4, cc * 128 : (cc + 1) * 128],
                    rhs=wout_sb[:, k4, 512:1024],
                    start=(k4 == 0),
                    stop=(k4 == 3),
                )
            nc.vector.tensor_copy(out=o_sb[:, 512:1024], in_=po2[:])
            nc.sync.dma_start(out=out[c * 128 : (c + 1) * 128, :], in_=o_sb[:])
        for cc in (2, 3):
            c = 12 + cc
            o_sb = outp.tile([128, D], mybir.dt.bfloat16, tag="osb", name=f"o_{c}")
            for half in range(2):
                po = ps_m.tile(
                    [128, 512], mybir.dt.float32, tag="misc", name=f"po_{c}_{half}"
                )
                for k4 in range(4):
                    nc.tensor.matmul(
                        po[:],
                        lhsT=waT_ring[3][:, k4, cc * 128 : (cc + 1) * 128],
                        rhs=wout_sb[:, k4, half * 512 : (half + 1) * 512],
                        start=(k4 == 0),
                        stop=(k4 == 3),
                    )
                dst = o_sb[:, half * 512 : (half + 1) * 512]
                if half == 0:
                    nc.scalar.activation(
                        out=dst, in_=po[:], func=mybir.ActivationFunctionType.Copy
                    )
                else:
                    nc.vector.tensor_copy(out=dst, in_=po[:])
            nc.sync.dma_start(out=out[c * 128 : (c + 1) * 128, :], in_=o_sb[:])

    nc.compile()
    return nc


def _prep_in_maps(x, w_qkv, b_qkv, w_out):
    """Host-side shard + relayout. Core c -> (batch c//2, head-group c%2)."""
    wq = w_qkv[:, :D].reshape(D, H, HD)
    wk = w_qkv[:, D : 2 * D].reshape(D, H, HD)
    wv_ = w_qkv[:, 2 * D :].reshape(D, H, HD)
    bq = b_qkv[:D].reshape(H, HD)
    bk = b_qkv[D : 2 * D].reshape(H, HD)
    bv = b_qkv[2 * D :].reshape(H, HD)
    wo = w_out.reshape(H, HD, D)

    per_group = {}
    for g in range(G):
        h0 = g * LH
        # feature order: block t=0 = k feats, t=1 = q feats (scaled by
        # QSCALE so the DVE bit-trick exp needs no multiply); within a
        # block, pair p occupies cols p*128..(p+1)*128 (first head at 0-63).
        Wqk = np.empty((D, 2, 4, 128), F32)
        Bqk = np.empty((2, 4, 128), F32)
        for p in range(LH // 2):
            ha, hb = h0 + 2 * p, h0 + 2 * p + 1
            Wqk[:, 0, p, 0:64] = wk[:, ha]
            Wqk[:, 0, p, 64:128] = wk[:, hb]
            Wqk[:, 1, p, 0:64] = wq[:, ha] * QSCALE
            Wqk[:, 1, p, 64:128] = wq[:, hb] * QSCALE
            Bqk[0, p, 0:64] = bk[ha]
            Bqk[0, p, 64:128] = bk[hb]
            Bqk[1, p, 0:64] = bq[ha] * QSCALE
            Bqk[1, p, 64:128] = bq[hb] * QSCALE
        wqk_arr = np.ascontiguousarray(
            Wqk.reshape(DC, 128, 2, 512).transpose(1, 0, 2, 3)
        ).astype(BF16)
        bqk_arr = np.ascontiguousarray(Bqk.reshape(8, 128).T)

        Wv = wv_[:, h0 : h0 + LH, :].reshape(D, LH * HD)
        wv_arr = np.ascontiguousarray(
            Wv.reshape(DC, 128, LH * HD).transpose(1, 0, 2)
        ).astype(BF16)

        Wo = wo[h0 : h0 + LH].reshape(LH * HD, D)
        wout_arr = np.ascontiguousarray(
            Wo.reshape(LH * HD // 128, 128, D).transpose(1, 0, 2)
        ).astype(BF16)
        per_group[g] = (wqk_arr, bqk_arr, wv_arr, wout_arr)

    in_maps = []
    for c in range(NCORES):
        b, g = divmod(c, G)
        wqk_arr, bqk_arr, wv_arr, wout_arr = per_group[g]
        xT_arr = np.ascontiguousarray(
            x[b].T.reshape(DC, 128, N).transpose(1, 0, 2)
        ).astype(BF16)
        in_maps.append(
            {
                "xT": xT_arr,
                "wqk": wqk_arr,
                "bqk": bqk_arr,
                "wv": wv_arr,
                "wout": wout_arr,
            }
        )
    return in_maps


def _ensure_ntff_hook():
    """Register the axon NTFF profile hook if the image's antenv lacks it.

    Mirrors trn_agent_boot.trn_boot._ntff_profile_via_ctypes: drives NRT
    profiling through the injected libaxon_pjrt.so C ABI. Without this,
    run_bass_kernel_spmd(trace=True) raises ImportError under axon.
    """
    try:
        from antenv.axon_hooks import get_axon_ntff_profile_hook  # noqa: F401

        return
    except ImportError:
        pass

    import contextlib
    import ctypes
    import types

    so_path = "/opt/axon/libaxon_pjrt.so"
    lib = ctypes.CDLL(so_path)
    if not hasattr(lib, "axon_start_nrt_profile"):
        return
    lib.axon_start_nrt_profile.argtypes = [ctypes.POINTER(ctypes.c_int64), ctypes.c_size_t]
    lib.axon_start_nrt_profile.restype = ctypes.c_int64
    lib.axon_stop_nrt_profile.argtypes = [ctypes.c_char_p]
    lib.axon_stop_nrt_profile.restype = ctypes.c_int64

    @contextlib.contextmanager
    def _hook(output_dir, device_ids):
        import jax

        jax.devices()
        if device_ids:
            ids = (ctypes.c_int64 * len(device_ids))(*device_ids)
            rc = lib.axon_start_nrt_profile(ids, len(device_ids))
        else:
            rc = lib.axon_start_nrt_profile(None, 0)
        if rc != 0:
            raise RuntimeError(f"axon_start_nrt_profile rc={rc}")
        try:
            yield
        finally:
            n = lib.axon_stop_nrt_profile(str(output_dir).encode())
            print(f"ntff profile: {n} file(s) written to {output_dir}", file=sys.stderr)

    mod = types.ModuleType("antenv.axon_hooks")
    mod.get_axon_ntff_profile_hook = lambda: _hook
    sys.modules["antenv.axon_hooks"] = mod

    # No artifact bucket in this sandbox; keep the NEFF dir local.
    from concourse import bass_utils as _bu

    _bu.upload_artifacts = lambda tmpdir: tmpdir


def kernel(x, w_qkv, b_qkv, w_out, b_out):
    x = np.asarray(x, dtype=F32)
    w_qkv = np.asarray(w_qkv, dtype=F32)
    b_qkv = np.asarray(b_qkv, dtype=F32)
    w_out = np.asarray(w_out, dtype=F32)
    b_out = np.asarray(b_out, dtype=F32)

    if "nc" not in _CACHE:
        _CACHE["nc"] = _build_nc()
    nc = _CACHE["nc"]

    in_maps = _prep_in_maps(x, w_qkv, b_qkv, w_out)
    trace = bool(int(os.environ.get("BASSMHA_TRACE", "0")))
    kwargs = {}
    if trace:
        _ensure_ntff_hook()
        tdir = os.environ.get("BASSMHA_TRACE_DIR")
        if tdir:
            os.makedirs(tdir, exist_ok=True)
            kwargs["tmpdir"] = tdir
    res = run_bass_kernel_spmd(nc, in_maps, list(range(NCORES)), trace=trace, **kwargs)
    _CACHE["last_results"] = res

    # v-bias folds exactly into an output constant: wa @ (v + bv) =
    # wa @ v + bv because the attention weights sum to 1.
    const_add = b_out + b_qkv[2 * D :].astype(F32) @ w_out
    out = np.empty((B, N, D), F32)
    for b in range(B):
        out[b] = res.results[2 * b]["out"].astype(F32)
        out[b] += res.results[2 * b + 1]["out"].astype(F32)
        out[b] += const_add
    return out


# revision 26
# speedup vs baseline: 1.0058x; 1.0058x over previous
"""Multi-head self-attention on 8 Trainium2 NeuronCores.

Problem: x[4, 2048, 1024], 16 heads x 64 dims, fused qkv + attention + out-proj.

Sharding (hybrid, per the tensor-parallel hint): core c handles batch b = c//2
and head-group g = c%2 (8 of the 16 heads). Each core computes a partial
out-projection over its 8 heads; the host sums the two group partials per
batch and adds b_out.

Per-core kernel (all matmuls bf16, fp32 PSUM accumulation):
  - exp is SPLIT between the scalar engine (true exp out of PSUM) and the
    vector engine (Schraudolph bit-trick: i16 = trunc(psc + 16249) bitcast
    to bf16 ~= exp(s/8); the multiply 23.083*s is folded into the HOST-side
    q-projection weights so the DVE op is a single tensor_scalar_add - the
    fused two-op mult+add with int16 output drops products on real HW).
    7 of 16 k-chunks go to DVE; this halves the ACT time per unit, which
    was the baseline's critical path (ACT ~19us/unit vs PE ~13.7us/unit).
  - deep software pipeline: scores/exp of unit n+3 run during the AV of
    unit n (E tiles triple-buffered; the E ring shares its 32KB slots with
    xT, which dies once the prologue projections finish). The psum scores
    tile recycle (bufs=2) then references exp work from a full iteration
    ago, so the PE never stalls on ACT/DVE at unit boundaries and the HAM
    clock stays warm.
  - normalize is split: one ACT copy retires the pw PSUM slot into SBUF at
    the end of each iteration (plus ln + exp(-x) reciprocal on the pinned
    exp/ln table), then next iteration a GPSIMD partition_broadcast fans
    the reciprocal across 64 partitions and one DVE 2x-mode multiply writes
    the normalized waT. No PE involvement at all.
  - scores computed transposed (S^T[k, q] = kT.T @ qT) per 128-row k-chunk,
    with the two heads of a pair row-packed on disjoint PE row groups.
  - softmax denominator comes free as an all-ones column appended to V in
    the AV matmul.
  - the first 6 k-proj tiles run contraction-outer across idle PSUM banks
    so the PE tracks the arriving x DMA instead of idling per chunk.
  - qk-proj bias is applied by the ACT engine (Identity+bias PSUM->SBUF
    move) during the projection prologue where ACT is otherwise idle; the
    v-bias is folded EXACTLY into the host-side output constant
    (wa @ (v + bv) = wa @ v + bv since attention weights sum to 1).
  - out-proj staged in bf16 (halves the output DMA); host sums the two
    head-group partials in fp32. The final q-range's out-proj partially
    accumulates during the last AV chain to shorten the tail.
"""

import os
import sys
from contextlib import ExitStack

import numpy as np

for _p in ("/opt/trn_rl_repo",):
    if _p not in sys.path and os.path.isdir(_p):
        sys.path.insert(0, _p)

import ml_dtypes

import concourse.bass as bass
import concourse.tile as tile
from concourse import bacc, mybir
from concourse.bass_utils import run_bass_kernel_spmd

BF16 = ml_dtypes.bfloat16
F32 = np.float32

D = 1024
H = 16
HD = 64
B = 4
N = 2048
NCORES = 8
G = 2  # head groups (tensor-parallel axis)
LH = H // G  # local heads per core
DC = D // 128  # 8 contraction chunks
KC = N // 128  # 16 k-token chunks
QT = N // 512  # 4 q tiles
TOK = N // 128  # 16 token chunks

# k-chunks whose exp runs on DVE (Schraudolph bit-trick) instead of ACT
DVE_SET = frozenset((1, 3, 5, 8, 10, 12, 14))
QSCALE = float(0.125 * 128.0 / np.log(2.0))  # folded into host q weights
EXP_B = 16249.0  # bf16 exponent bias + rms-optimal shift (trunc-calibrated)
ACT_SCALE = float(0.125 / QSCALE)  # undo the fold for the true-exp path

_CACHE = {}


def _pin_act_tables():
    """Make the act-table chooser resolve exp AND ln to the one set that
    holds both (natural_log_exp_and_others), instead of thrashing between
    exp_and_others and natural_log on every softmax/reciprocal boundary
    (~1.3us ACT stall per reload). Other sets keep their index/id; we only
    hide exp/ln from them so they are never chosen for those funcs.
    """
    if _CACHE.get("act_pinned"):
        return
    from concourse import bacc as _bacc
    from concourse import hw_specs as _hw

    orig = _hw.get_activation_tables

    def patched(arch):
        t = dict(orig(arch))
        keep = "natural_log_exp_and_others"
        if keep in t:
            pinned = t[keep]
            t = {n: (s if n == keep else (s - pinned)) for n, s in t.items()}
        return t

    _hw.get_activation_tables = patched
    _bacc.get_activation_tables = patched
    _CACHE["act_pinned"] = True


def _build_nc():
    _pin_act_tables()
    nc = bacc.Bacc(None, target_bir_lowering=False)

    xT = nc.declare_dram_parameter("xT", [128, DC, N], mybir.dt.bfloat16, isOutput=False)
    # wqk[:, kc, 0, :] = k-features (4 pairs x 128), [:, kc, 1, :] = q-features
    wqk = nc.declare_dram_parameter("wqk", [128, DC, 2, 512], mybir.dt.bfloat16, isOutput=False)
    bqk = nc.declare_dram_parameter("bqk", [128, 8], mybir.dt.float32, isOutput=False)
    wv = nc.declare_dram_parameter("wv", [128, DC, LH * HD], mybir.dt.bfloat16, isOutput=False)
    wout = nc.declare_dram_parameter("wout", [128, LH * HD // 128, D], mybir.dt.bfloat16, isOutput=False)
    out = nc.declare_dram_parameter("out", [N, D], mybir.dt.bfloat16, isOutput=True)

    with tile.TileContext(nc) as tc, ExitStack() as ctx:
        const = ctx.enter_context(tc.tile_pool(name="const", bufs=1))
        big = ctx.enter_context(tc.tile_pool(name="big", bufs=4))
        work = ctx.enter_context(tc.tile_pool(name="work", bufs=1))
        outp = ctx.enter_context(tc.tile_pool(name="outp", bufs=2))
        small = ctx.enter_context(tc.tile_pool(name="small", bufs=2))
        ps_s = ctx.enter_context(tc.tile_pool(name="ps_s", bufs=2, space="PSUM"))
        ps_wa = ctx.enter_context(tc.tile_pool(name="ps_wa", bufs=2, space="PSUM"))
        ps_m = ctx.enter_context(tc.tile_pool(name="ps_m", bufs=2, space="PSUM"))

        bqk_sb = const.tile([128, 8], mybir.dt.float32)
        # xT, wqk and wv share the 32KB big-pool ring with the E tiles:
        # they occupy slots during the projection prologue and are recycled
        # by E(1), E(2) and E(3) once their last readers retire.
        xT_sb = big.tile([128, DC, N], mybir.dt.bfloat16, tag="big", name="xT")
        wqk_sb = big.tile([128, DC, 2, 512], mybir.dt.bfloat16, tag="big", name="wqk")
        wv_sb = big.tile([128, DC, LH * HD], mybir.dt.bfloat16, tag="big", name="wv")
        nc.sync.dma_start(out=bqk_sb[:], in_=bqk[:])
        wout_sb = const.tile([128, LH * HD // 128, D], mybir.dt.bfloat16)
        # qkT[:, 0, p, :] = k-features of pair p; [:, 1, p, :] = q-features
        qkT_sb = work.tile([128, 2, 4, N], mybir.dt.bfloat16, tag="qkT")
        V_sb = work.tile([128, KC, LH, HD + 1], mybir.dt.bfloat16, tag="V")
        wa_pool = ctx.enter_context(tc.tile_pool(name="wa_pool", bufs=2))
        waT_ring = {}

        # ones column (index HD) for the free softmax denominator; the v-proj
        # copies below only fill [0:HD] so the column survives.
        nc.vector.memset(V_sb[:, :, :, HD : HD + 1], 1.0)

        def emit_proj(t, p, tt):
            """One [128 feats, 512 toks] tile of the q/k projection.
            t=0 -> k-features, t=1 -> q-features of pair p, token tile tt.
            PSUM->SBUF move with per-feature bias runs on ACT (idle here)."""
            pq = ps_m.tile([128, 512], mybir.dt.float32, tag="misc", name=f"pq_{t}_{p}_{tt}")
            for kc in range(DC):
                nc.tensor.matmul(
                    pq[:],
                    lhsT=wqk_sb[:, kc, t, p * 128 : (p + 1) * 128],
                    rhs=xT_sb[:, kc, tt * 512 : (tt + 1) * 512],
                    start=(kc == 0),
                    stop=(kc == DC - 1),
                )
            nc.scalar.activation(
                out=qkT_sb[:, t, p, tt * 512 : (tt + 1) * 512],
                in_=pq[:],
                func=mybir.ActivationFunctionType.Identity,
                bias=bqk_sb[:, t * 4 + p : t * 4 + p + 1],
            )

        def emit_vproj():
            # v projection: V[tok, feat] = x @ w_v. The v-bias is EXACT as a
            # host-side output constant (attention weights sum to 1, so
            # wa @ (v + bv) = wa @ v + bv): folded into b_out on the host.
            for c in range(TOK):
                pv = ps_m.tile([128, 512], mybir.dt.float32, tag="misc", name=f"pv_{c}")
                for kc in range(DC):
                    nc.tensor.matmul(
                        pv[:],
                        lhsT=xT_sb[:, kc, c * 128 : (c + 1) * 128],
                        rhs=wv_sb[:, kc, :],
                        start=(kc == 0),
                        stop=(kc == DC - 1),
                    )
                nc.vector.tensor_copy(
                    out=V_sb[:, c, :, 0:HD],
                    in_=pv[:].rearrange("p (l d) -> p l d", l=LH),
                )

        def unit(n):
            return n // 4, n % 4  # (q4, pair)

        def emit_scores_chunk(n, kc, E):
            q4, pair = unit(n)
            psc = ps_s.tile([128, 1024], mybir.dt.float32, tag="sc", name=f"sc_{n}_{kc}")
            for h01 in range(2):
                row = 64 * h01
                nc.tensor.matmul(
                    psc[:, h01 * 512 : (h01 + 1) * 512],
                    lhsT=qkT_sb[row : row + 64, 0, pair, kc * 128 : (kc + 1) * 128],
                    rhs=qkT_sb[row : row + 64, 1, pair, q4 * 512 : (q4 + 1) * 512],
                    start=True,
                    stop=True,
                )
            if kc in DVE_SET:
                # q-weights carry the 23.083 scale: one add + int16 convert
                # IS exp (Schraudolph), bitcast back to bf16 via the E view.
                nc.vector.tensor_scalar_add(
                    out=E[:, kc, :].bitcast(mybir.dt.int16),
                    in0=psc[:],
                    scalar1=EXP_B,
                )
            else:
                nc.scalar.activation(
                    out=E[:, kc, :],
                    in_=psc[:],
                    func=mybir.ActivationFunctionType.Exp,
                    scale=ACT_SCALE,
                )

        def emit_av_chunk(n, kc, E, pw, hs=(0, 1)):
            _, pair = unit(n)
            for h01 in hs:
                nc.tensor.matmul(
                    pw[h01][:],
                    lhsT=V_sb[:, kc, 2 * pair + h01, :],
                    rhs=E[:, kc, h01 * 512 : (h01 + 1) * 512],
                    start=(kc == 0),
                    stop=(kc == KC - 1),
                )

        def emit_norm_a(n, h01, pw):
            """Phase A (emitted at the end of unit n's iteration): one ACT
            copy retires the pw PSUM slot into SBUF so the next unit's AV
            chain can recycle it immediately, then ln + exp(-x) compute the
            reciprocal of the denominator row (same pinned act table as the
            softmax exp)."""
            pwS = small.tile([65, 512], mybir.dt.bfloat16, tag="pwS", name=f"pwS_{n}_{h01}")
            nc.scalar.activation(
                out=pwS[:], in_=pw[:], func=mybir.ActivationFunctionType.Copy
            )
            lg = small.tile([1, 512], mybir.dt.bfloat16, tag="lg", name=f"lg_{n}_{h01}")
            nc.scalar.activation(
                out=lg[:], in_=pwS[64:65, :], func=mybir.ActivationFunctionType.Ln
            )
            recip = small.tile([1, 512], mybir.dt.bfloat16, tag="recip", name=f"r_{n}_{h01}")
            nc.scalar.activation(
                out=recip[:],
                in_=lg[:],
                func=mybir.ActivationFunctionType.Exp,
                scale=-1.0,
            )
            return pwS, recip

        def emit_norm_b_pb(n, h01, recip):
            """Phase B part 1 (next iteration): broadcast the reciprocal
            across 64 partitions on the otherwise-idle GPSIMD engine (frees
            the PE of 32 rank-1 matmuls and their stationary-swap drains)."""
            rb = small.tile([64, 512], mybir.dt.bfloat16, tag="rb", name=f"rb_{n}_{h01}")
            nc.gpsimd.partition_broadcast(rb[:], recip[:])
            return rb

        def emit_norm_b_mul(n, h01, pwS, rb):
            """Phase B part 2 (DVE): normalized waT = wa * (1/denom); both
            operands SBUF bf16 so the DVE runs in its 2x mode."""
            q4, pair = unit(n)
            row = 64 * h01
            nc.vector.tensor_mul(
                out=waT_ring[q4][row : row + 64, pair, :],
                in0=pwS[0:64, :],
                in1=rb[:],
            )

        def emit_outproj(oq4):
            # out projection for a finished q-range; overlaps the next
            # q-range's attention stream. PSUM->SBUF moves alternate between
            # ACT and DVE to split the load; staged/stored in bf16.
            for cc in range(4):
                c = oq4 * 4 + cc
                o_sb = outp.tile([128, D], mybir.dt.bfloat16, tag="osb", name=f"o_{c}")
                for half in range(2):
                    po = ps_m.tile(
                        [128, 512], mybir.dt.float32, tag="misc", name=f"po_{c}_{half}"
                    )
                    for k4 in range(LH * HD // 128):
                        nc.tensor.matmul(
                            po[:],
                            lhsT=waT_ring[oq4][:, k4, cc * 128 : (cc + 1) * 128],
                            rhs=wout_sb[:, k4, half * 512 : (half + 1) * 512],
                            start=(k4 == 0),
                            stop=(k4 == LH * HD // 128 - 1),
                        )
                    dst = o_sb[:, half * 512 : (half + 1) * 512]
                    if half == 0:
                        nc.scalar.activation(
                            out=dst, in_=po[:], func=mybir.ActivationFunctionType.Copy
                        )
                    else:
                        nc.vector.tensor_copy(out=dst, in_=po[:])
                nc.sync.dma_start(out=out[c * 128 : (c + 1) * 128, :], in_=o_sb[:])

        # ---- prologue: all projections, then prime 3 units of scores ----
        # First 6 k-proj tiles run contraction-OUTER across idle PSUM banks
        # so each arriving xT chunk immediately feeds 6 matmuls and the PE
        # tracks the input DMA instead of idling ~1.4us per chunk.
        ko_sc = [
            ps_s.tile([128, 1024], mybir.dt.float32, tag="sc", name=f"ko_sc{i}")
            for i in range(2)
        ]
        ko_m = [
            ps_m.tile([128, 512], mybir.dt.float32, tag="misc", name=f"ko_m{i}")
            for i in range(2)
        ]
        ko_slots = [
            (ko_sc[0][:, 0:512], 0, 0), (ko_sc[0][:, 512:1024], 0, 1),
            (ko_sc[1][:, 0:512], 0, 2), (ko_sc[1][:, 512:1024], 0, 3),
            (ko_m[0][:], 1, 0), (ko_m[1][:], 1, 1),
        ]
        # DMA issues interleaved with the matmuls that consume them, so each
        # chunk's matmuls gate on just that chunk's two transfers instead of
        # a coarse all-inputs semaphore threshold.
        for kc in range(DC):
            nc.sync.dma_start(out=xT_sb[:, kc, :], in_=xT[:, kc, :])
            nc.sync.dma_start(out=wqk_sb[:, kc, 0, :], in_=wqk[:, kc, 0, :])
            for dst, p, tt in ko_slots:
                nc.tensor.matmul(
                    dst,
                    lhsT=wqk_sb[:, kc, 0, p * 128 : (p + 1) * 128],
                    rhs=xT_sb[:, kc, tt * 512 : (tt + 1) * 512],
                    start=(kc == 0),
                    stop=(kc == DC - 1),
                )
        for kc in range(DC):
            nc.sync.dma_start(out=wqk_sb[:, kc, 1, :], in_=wqk[:, kc, 1, :])
        nc.sync.dma_start(out=wv_sb[:], in_=wv[:])
        nc.sync.dma_start(out=wout_sb[:], in_=wout[:])
        for dst, p, tt in ko_slots:
            nc.scalar.activation(
                out=qkT_sb[:, 0, p, tt * 512 : (tt + 1) * 512],
                in_=dst,
                func=mybir.ActivationFunctionType.Identity,
                bias=bqk_sb[:, p : p + 1],
            )
        for p, tt in [(1, 2), (1, 3), (2, 0), (2, 1), (2, 2), (2, 3), (3, 0), (3, 1), (3, 2), (3, 3)]:
            emit_proj(0, p, tt)
        for p in range(4):
            emit_proj(1, p, 0)

        E_tiles = {}

        def alloc_E(n):
            E_tiles[n] = big.tile(
                [128, KC, 1024], mybir.dt.bfloat16, tag="big", name=f"E_{n}"
            )

        for n in (0, 1):
            alloc_E(n)
            for kc in range(KC):
                emit_scores_chunk(n, kc, E_tiles[n])
        for p in range(4):
            for tt in range(1, QT):
                emit_proj(1, p, tt)
        emit_vproj()  # last reader of xT; E(2) recycles its big-pool slot
        alloc_E(2)
        for kc in range(KC):
            emit_scores_chunk(2, kc, E_tiles[2])

        # ---- steady state: AV(n) interleaved with scores/exp(n+3) ----
        # Emission order per iteration is tuned so every cross-engine gate
        # (pw-slot recycle, psc-slot recycle, E-slot recycle) is satisfied
        # ~an iteration before the PE reaches the dependent instruction.
        LEAD = 3
        FRONT = 5
        norm_prev = None  # (pwS, recip) pairs of unit n-1
        pw_prev = None
        for n in range(15):
            m = n + LEAD
            if m <= 15:
                alloc_E(m)
            pw = [
                ps_wa.tile([65, 512], mybir.dt.float32, tag="wa", name=f"wa_{n}_{h}")
                for h in range(2)
            ]
            for kc in range(FRONT):
                emit_av_chunk(n, kc, E_tiles[n], pw)
            if m <= 15:
                for kc in range(FRONT):
                    emit_scores_chunk(m, kc, E_tiles[m])
            pbs = None
            if n >= 1:
                if (n - 1) % 4 == 0:
                    waT_ring[(n - 1) // 4] = wa_pool.tile(
                        [128, LH * HD // 128, 512], mybir.dt.bfloat16,
                        tag="waT", name=f"waT_{(n - 1) // 4}",
                    )
                pbs = [emit_norm_b_pb(n - 1, h, norm_prev[h][1]) for h in range(2)]
            for idx, j in enumerate(range(FRONT, KC - 1, 2)):
                emit_av_chunk(n, j, E_tiles[n], pw)
                emit_av_chunk(n, j + 1, E_tiles[n], pw)
                if m <= 15:
                    emit_scores_chunk(m, j, E_tiles[m])
                    emit_scores_chunk(m, j + 1, E_tiles[m])
                if idx == 1:
                    # muls late in the DVE queue (the exp chunks that gate
                    # the psc recycle come first); outproj right after the
                    # muls it depends on.
                    if pbs is not None:
                        for h01 in range(2):
                            emit_norm_b_mul(n - 1, h01, norm_prev[h01][0], pbs[h01])
                    if n % 4 == 0 and n > 0:
                        emit_outproj(n // 4 - 1)
            emit_av_chunk(n, KC - 1, E_tiles[n], pw)
            norm_prev = [emit_norm_a(n, h01, pw[h01]) for h01 in range(2)]
            if m <= 15:
                emit_scores_chunk(m, KC - 1, E_tiles[m])
            pw_prev = pw
            del E_tiles[n]
        # ---- unit 15: h0 chain -> its normalize overlaps the h1 chain, so
        # outproj(3) only waits on the short h1 normalize tail ----
        bcs14 = [emit_norm_b_pb(14, h, norm_prev[h][1]) for h in range(2)]
        pw = [
            ps_wa.tile([65, 512], mybir.dt.float32, tag="wa", name=f"wa_15_{h}")
            for h in range(2)
        ]
        waT_ring[3] = waT_ring.get(3) or wa_pool.tile(
            [128, LH * HD // 128, 512], mybir.dt.bfloat16, tag="waT", name="waT_3"
        )
        for kc in range(2):
            emit_av_chunk(15, kc, E_tiles[15], pw, hs=(0,))
        for h01 in range(2):
            emit_norm_b_mul(14, h01, norm_prev[h01][0], bcs14[h01])
        for kc in range(2, KC):
            emit_av_chunk(15, kc, E_tiles[15], pw, hs=(0,))
        n15_h0 = emit_norm_a(15, 0, pw[0])
        bc15_0 = emit_norm_b_pb(15, 0, n15_h0[1])
        emit_norm_b_mul(15, 0, n15_h0[0], bc15_0)
        for kc in range(KC):
            emit_av_chunk(15, kc, E_tiles[15], pw, hs=(1,))
        # last normalize skips the pwS staging (nothing recycles pw after
        # this) so outproj's final matmuls wait ~1us less.
        lg15 = small.tile([1, 512], mybir.dt.bfloat16, tag="lg", name="lg_15_1")
        nc.scalar.activation(
            out=lg15[:], in_=pw[1][64:65, :], func=mybir.ActivationFunctionType.Ln
        )
        r15 = small.tile([1, 512], mybir.dt.bfloat16, tag="recip", name="r_15_1")
        nc.scalar.activation(
            out=r15[:], in_=lg15[:], func=mybir.ActivationFunctionType.Exp, scale=-1.0
        )
        rb15 = small.tile([64, 512], mybir.dt.bfloat16, tag="rb", name="rb_15_1")
        nc.gpsimd.partition_broadcast(rb15[:], r15[:])
        # partial outproj chains for the first two token chunks of q4=3:
        # pairs 0-2 are normalized, so k4 0..2 accumulate during the tail
        pos_part = {}
        for cc in (0, 1):
            po = ps_m.tile([128, 512], mybir.dt.float32, tag="misc", name=f"po_p_{cc}")
            for k4 in range(3):
                nc.tensor.matmul(
                    po[:],
                    lhsT=waT_ring[3][:, k4, cc * 128 : (cc + 1) * 128],
                    rhs=wout_sb[:, k4, 0:512] if False else wout_sb[:, k4, :][:, 0:1024][:, 0:512],
                    start=(k4 == 0),
                    stop=False,
                )
            pos_part[cc] = po
        nc.vector.tensor_mul(
            out=waT_ring[3][64:128, 3, :], in0=pw[1][0:64, :], in1=rb15[:]
        )
        del E_tiles[15]
        # finish the partial chains (k4=3 needs unit 15's normalize), then
        # the remaining chunks
        for cc in (0, 1):
            c = 12 + cc
            o_sb = outp.tile([128, D], mybir.dt.bfloat16, tag="osb", name=f"o_{c}")
            po = pos_part[cc]
            nc.tensor.matmul(
                po[:],
                lhsT=waT_ring[3][:, 3, cc * 128 : (cc + 1) * 128],
                rhs=wout_sb[:, 3, 0:512],
                start=False,
                stop=True,
            )
            nc.scalar.activation(
                out=o_sb[:, 0:512], in_=po[:], func=mybir.ActivationFunctionType.Copy
            )
            po2 = ps_m.tile([128, 512], mybir.dt.float32, tag="misc", name=f"po_p2_{cc}")
            for k4 in range(4):
                nc.tensor.matmul(
                    po2[:],
                    lhsT=waT_ring[3][:, k4, cc * 128 : (cc + 1) * 128],
                    rhs=wout_sb[:, k4, 512:1024],
                    start=(k4 == 0),
                    stop=(k4 == 3),
                )
            nc.vector.tensor_copy(out=o_sb[:, 512:1024], in_=po2[:])
            nc.sync.dma_start(out=out[c * 128 : (c + 1) * 128, :], in_=o_sb[:])
        for cc in (2, 3):
            c = 12 + cc
            o_sb = outp.tile([128, D], mybir.dt.bfloat16, tag="osb", name=f"o_{c}")
            for half in range(2):
                po = ps_m.tile(
                    [128, 512], mybir.dt.float32, tag="misc", name=f"po_{c}_{half}"
                )
                for k4 in range(4):
                    nc.tensor.matmul(
                        po[:],
                        lhsT=waT_ring[3][:, k4, cc * 128 : (cc + 1) * 128],
                        rhs=wout_sb[:, k4, half * 512 : (half + 1) * 512],
                        start=(k4 == 0),
                        stop=(k4 == 3),
                    )
                dst = o_sb[:, half * 512 : (half + 1) * 512]
                if half == 0:
                    nc.scalar.activation(
                        out=dst, in_=po[:], func=mybir.ActivationFunctionType.Copy
                    )
                else:
                    nc.vector.tensor_copy(out=dst, in_=po[:])
            nc.sync.dma_start(out=out[c * 128 : (c + 1) * 128, :], in_=o_sb[:])

    nc.compile()
    return nc


def _prep_in_maps(x, w_qkv, b_qkv, w_out):
    """Host-side shard + relayout. Core c -> (batch c//2, head-group c%2)."""
    wq = w_qkv[:, :D].reshape(D, H, HD)
    wk = w_qkv[:, D : 2 * D].reshape(D, H, HD)
    wv_ = w_qkv[:, 2 * D :].reshape(D, H, HD)
    bq = b_qkv[:D].reshape(H, HD)
    bk = b_qkv[D : 2 * D].reshape(H, HD)
    bv = b_qkv[2 * D :].reshape(H, HD)
    wo = w_out.reshape(H, HD, D)

    per_group = {}
    for g in range(G):
        h0 = g * LH
        # feature order: block t=0 = k feats, t=1 = q feats (scaled by
        # QSCALE so the DVE bit-trick exp needs no multiply); within a
        # block, pair p occupies cols p*128..(p+1)*128 (first head at 0-63).
        Wqk = np.empty((D, 2, 4, 128), F32)
        Bqk = np.empty((2, 4, 128), F32)
        for p in range(LH // 2):
            ha, hb = h0 + 2 * p, h0 + 2 * p + 1
            Wqk[:, 0, p, 0:64] = wk[:, ha]
            Wqk[:, 0, p, 64:128] = wk[:, hb]
            Wqk[:, 1, p, 0:64] = wq[:, ha] * QSCALE
            Wqk[:, 1, p, 64:128] = wq[:, hb] * QSCALE
            Bqk[0, p, 0:64] = bk[ha]
            Bqk[0, p, 64:128] = bk[hb]
            Bqk[1, p, 0:64] = bq[ha] * QSCALE
            Bqk[1, p, 64:128] = bq[hb] * QSCALE
        wqk_arr = np.ascontiguousarray(
            Wqk.reshape(DC, 128, 2, 512).transpose(1, 0, 2, 3)
        ).astype(BF16)
        bqk_arr = np.ascontiguousarray(Bqk.reshape(8, 128).T)

        Wv = wv_[:, h0 : h0 + LH, :].reshape(D, LH * HD)
        wv_arr = np.ascontiguousarray(
            Wv.reshape(DC, 128, LH * HD).transpose(1, 0, 2)
        ).astype(BF16)

        Wo = wo[h0 : h0 + LH].reshape(LH * HD, D)
        wout_arr = np.ascontiguousarray(
            Wo.reshape(LH * HD // 128, 128, D).transpose(1, 0, 2)
        ).astype(BF16)
        per_group[g] = (wqk_arr, bqk_arr, wv_arr, wout_arr)

    in_maps = []
    for c in range(NCORES):
        b, g = divmod(c, G)
        wqk_arr, bqk_arr, wv_arr, wout_arr = per_group[g]
        xT_arr = np.ascontiguousarray(
            x[b].T.reshape(DC, 128, N).transpose(1, 0, 2)
        ).astype(BF16)
        in_maps.append(
            {
                "xT": xT_arr,
                "wqk": wqk_arr,
                "bqk": bqk_arr,
                "wv": wv_arr,
                "wout": wout_arr,
            }
        )
    return in_maps


def _ensure_ntff_hook():
    """Register the axon NTFF profile hook if the image's antenv lacks it.

    Mirrors trn_agent_boot.trn_boot._ntff_profile_via_ctypes: drives NRT
    profiling through the injected libaxon_pjrt.so C ABI. Without this,
    run_bass_kernel_spmd(trace=True) raises ImportError under axon.
    """
    try:
        from antenv.axon_hooks import get_axon_ntff_profile_hook  # noqa: F401

        return
    except ImportError:
        pass

    import contextlib
    import ctypes
    import types

    so_path = "/opt/axon/libaxon_pjrt.so"
    lib = ctypes.CDLL(so_path)
    if not hasattr(lib, "axon_start_nrt_profile"):
        return
    lib.axon_start_nrt_profile.argtypes = [ctypes.POINTER(ctypes.c_int64), ctypes.c_size_t]
    lib.axon_start_nrt_profile.restype = ctypes.c_int64
    lib.axon_stop_nrt_profile.argtypes = [ctypes.c_char_p]
    lib.axon_stop_nrt_profile.restype = ctypes.c_int64

    @contextlib.contextmanager
    def _hook(output_dir, device_ids):
        import jax

        jax.devices()
        if device_ids:
            ids = (ctypes.c_int64 * len(device_ids))(*device_ids)
            rc = lib.axon_start_nrt_profile(ids, len(device_ids))
        else:
            rc = lib.axon_start_nrt_profile(None, 0)
        if rc != 0:
            raise RuntimeError(f"axon_start_nrt_profile rc={rc}")
        try:
            yield
        finally:
            n = lib.axon_stop_nrt_profile(str(output_dir).encode())
            print(f"ntff profile: {n} file(s) written to {output_dir}", file=sys.stderr)

    mod = types.ModuleType("antenv.axon_hooks")
    mod.get_axon_ntff_profile_hook = lambda: _hook
    sys.modules["antenv.axon_hooks"] = mod

    # No artifact bucket in this sandbox; keep the NEFF dir local.
    from concourse import bass_utils as _bu

    _bu.upload_artifacts = lambda tmpdir: tmpdir


def kernel(x, w_qkv, b_qkv, w_out, b_out):
    x = np.asarray(x, dtype=F32)
    w_qkv = np.asarray(w_qkv, dtype=F32)
    b_qkv = np.asarray(b_qkv, dtype=F32)
    w_out = np.asarray(w_out, dtype=F32)
    b_out = np.asarray(b_out, dtype=F32)

    if "nc" not in _CACHE:
        _CACHE["nc"] = _build_nc()
    nc = _CACHE["nc"]

    in_maps = _prep_in_maps(x, w_qkv, b_qkv, w_out)
    trace = bool(int(os.environ.get("BASSMHA_TRACE", "0")))
    kwargs = {}
    if trace:
        _ensure_ntff_hook()
        tdir = os.environ.get("BASSMHA_TRACE_DIR")
        if tdir:
            os.makedirs(tdir, exist_ok=True)
            kwargs["tmpdir"] = tdir
    res = run_bass_kernel_spmd(nc, in_maps, list(range(NCORES)), trace=trace, **kwargs)
    _CACHE["last_results"] = res

    # v-bias folds exactly into an output constant: wa @ (v + bv) =
    # wa @ v + bv because the attention weights sum to 1.
    const_add = b_out + b_qkv[2 * D :].astype(F32) @ w_out
    out = np.empty((B, N, D), F32)
    for b in range(B):
        out[b] = res.results[2 * b]["out"].astype(F32)
        out[b] += res.results[2 * b + 1]["out"].astype(F32)
        out[b] += const_add
    return out


# revision 30
# speedup vs baseline: 1.0087x; 1.0030x over previous
"""Multi-head self-attention on 8 Trainium2 NeuronCores.

Problem: x[4, 2048, 1024], 16 heads x 64 dims, fused qkv + attention + out-proj.

Sharding (hybrid, per the tensor-parallel hint): core c handles batch b = c//2
and head-group g = c%2 (8 of the 16 heads). Each core computes a partial
out-projection over its 8 heads; the host sums the two group partials per
batch and adds b_out.

Per-core kernel (all matmuls bf16, fp32 PSUM accumulation):
  - exp is SPLIT between the scalar engine (true exp out of PSUM) and the
    vector engine (Schraudolph bit-trick: i16 = trunc(psc + 16249) bitcast
    to bf16 ~= exp(s/8); the multiply 23.083*s is folded into the HOST-side
    q-projection weights so the DVE op is a single tensor_scalar_add - the
    fused two-op mult+add with int16 output drops products on real HW).
    7 of 16 k-chunks go to DVE; this halves the ACT time per unit, which
    was the baseline's critical path (ACT ~19us/unit vs PE ~13.7us/unit).
  - deep software pipeline: scores/exp of unit n+3 run during the AV of
    unit n (E tiles triple-buffered; the E ring shares its 32KB slots with
    xT, which dies once the prologue projections finish). The psum scores
    tile recycle (bufs=2) then references exp work from a full iteration
    ago, so the PE never stalls on ACT/DVE at unit boundaries and the HAM
    clock stays warm.
  - normalize is split: one ACT copy retires the pw PSUM slot into SBUF at
    the end of each iteration (plus ln + exp(-x) reciprocal on the pinned
    exp/ln table), then next iteration a GPSIMD partition_broadcast fans
    the reciprocal across 64 partitions and one DVE 2x-mode multiply writes
    the normalized waT. No PE involvement at all.
  - scores computed transposed (S^T[k, q] = kT.T @ qT) per 128-row k-chunk,
    with the two heads of a pair row-packed on disjoint PE row groups.
  - softmax denominator comes free as an all-ones column appended to V in
    the AV matmul.
  - the first 6 k-proj tiles run contraction-outer across idle PSUM banks
    so the PE tracks the arriving x DMA instead of idling per chunk.
  - qk-proj bias is applied by the ACT engine (Identity+bias PSUM->SBUF
    move) during the projection prologue where ACT is otherwise idle; the
    v-bias is folded EXACTLY into the host-side output constant
    (wa @ (v + bv) = wa @ v + bv since attention weights sum to 1).
  - out-proj staged in bf16 (halves the output DMA); host sums the two
    head-group partials in fp32. The final q-range's out-proj partially
    accumulates during the last AV chain to shorten the tail.
"""

import os
import sys
from contextlib import ExitStack

import numpy as np

for _p in ("/opt/trn_rl_repo",):
    if _p not in sys.path and os.path.isdir(_p):
        sys.path.insert(0, _p)

import ml_dtypes

import concourse.bass as bass
import concourse.tile as tile
from concourse import bacc, mybir
from concourse.bass_utils import run_bass_kernel_spmd

BF16 = ml_dtypes.bfloat16
F32 = np.float32

D = 1024
H = 16
HD = 64
B = 4
N = 2048
NCORES = 8
G = 2  # head groups (tensor-parallel axis)
LH = H // G  # local heads per core
DC = D // 128  # 8 contraction chunks
KC = N // 128  # 16 k-token chunks
QT = N // 512  # 4 q tiles
TOK = N // 128  # 16 token chunks

# k-chunks whose exp runs on DVE (Schraudolph bit-trick) instead of ACT
DVE_SET = frozenset((1, 3, 5, 8, 10, 12, 14))
QSCALE = float(0.125 * 128.0 / np.log(2.0))  # folded into host q weights
EXP_B = 16249.0  # bf16 exponent bias + rms-optimal shift (trunc-calibrated)
ACT_SCALE = float(0.125 / QSCALE)  # undo the fold for the true-exp path

_CACHE = {}


def _pin_act_tables():
    """Make the act-table chooser resolve exp AND ln to the one set that
    holds both (natural_log_exp_and_others), instead of thrashing between
    exp_and_others and natural_log on every softmax/reciprocal boundary
    (~1.3us ACT stall per reload). Other sets keep their index/id; we only
    hide exp/ln from them so they are never chosen for those funcs.
    """
    if _CACHE.get("act_pinned"):
        return
    from concourse import bacc as _bacc
    from concourse import hw_specs as _hw

    orig = _hw.get_activation_tables

    def patched(arch):
        t = dict(orig(arch))
        keep = "natural_log_exp_and_others"
        if keep in t:
            pinned = t[keep]
            t = {n: (s if n == keep else (s - pinned)) for n, s in t.items()}
        return t

    _hw.get_activation_tables = patched
    _bacc.get_activation_tables = patched
    _CACHE["act_pinned"] = True


def _build_nc():
    _pin_act_tables()
    nc = bacc.Bacc(None, target_bir_lowering=False)

    xT = nc.declare_dram_parameter("xT", [128, DC, N], mybir.dt.bfloat16, isOutput=False)
    # wqk[:, kc, 0, :] = k-features (4 pairs x 128), [:, kc, 1, :] = q-features
    wqk = nc.declare_dram_parameter("wqk", [128, DC, 2, 512], mybir.dt.bfloat16, isOutput=False)
    bqk = nc.declare_dram_parameter("bqk", [128, 8], mybir.dt.float32, isOutput=False)
    wv = nc.declare_dram_parameter("wv", [128, DC, LH * HD], mybir.dt.bfloat16, isOutput=False)
    wout = nc.declare_dram_parameter("wout", [128, LH * HD // 128, D], mybir.dt.bfloat16, isOutput=False)
    out = nc.declare_dram_parameter("out", [N, D], mybir.dt.bfloat16, isOutput=True)

    with tile.TileContext(nc) as tc, ExitStack() as ctx:
        const = ctx.enter_context(tc.tile_pool(name="const", bufs=1))
        big = ctx.enter_context(tc.tile_pool(name="big", bufs=4))
        work = ctx.enter_context(tc.tile_pool(name="work", bufs=1))
        outp = ctx.enter_context(tc.tile_pool(name="outp", bufs=2))
        small = ctx.enter_context(tc.tile_pool(name="small", bufs=2))
        ps_s = ctx.enter_context(tc.tile_pool(name="ps_s", bufs=2, space="PSUM"))
        ps_wa = ctx.enter_context(tc.tile_pool(name="ps_wa", bufs=2, space="PSUM"))
        ps_m = ctx.enter_context(tc.tile_pool(name="ps_m", bufs=2, space="PSUM"))

        bqk_sb = const.tile([128, 8], mybir.dt.float32)
        # xT, wqk and wv share the 32KB big-pool ring with the E tiles:
        # they occupy slots during the projection prologue and are recycled
        # by E(1), E(2) and E(3) once their last readers retire.
        xT_sb = big.tile([128, DC, N], mybir.dt.bfloat16, tag="big", name="xT")
        wqk_sb = big.tile([128, DC, 2, 512], mybir.dt.bfloat16, tag="big", name="wqk")
        wv_sb = big.tile([128, DC, LH * HD], mybir.dt.bfloat16, tag="big", name="wv")
        nc.sync.dma_start(out=bqk_sb[:], in_=bqk[:])
        wout_sb = const.tile([128, LH * HD // 128, D], mybir.dt.bfloat16)
        # qkT[:, 0, p, :] = k-features of pair p; [:, 1, p, :] = q-features
        qkT_sb = work.tile([128, 2, 4, N], mybir.dt.bfloat16, tag="qkT")
        V_sb = work.tile([128, KC, LH, HD + 1], mybir.dt.bfloat16, tag="V")
        wa_pool = ctx.enter_context(tc.tile_pool(name="wa_pool", bufs=2))
        waT_ring = {}

        # ones column (index HD) for the free softmax denominator; the v-proj
        # copies below only fill [0:HD] so the column survives.
        nc.vector.memset(V_sb[:, :, :, HD : HD + 1], 1.0)

        def emit_proj(t, p, tt):
            """One [128 feats, 512 toks] tile of the q/k projection.
            t=0 -> k-features, t=1 -> q-features of pair p, token tile tt.
            PSUM->SBUF move with per-feature bias runs on ACT (idle here)."""
            pq = ps_m.tile([128, 512], mybir.dt.float32, tag="misc", name=f"pq_{t}_{p}_{tt}")
            for kc in range(DC):
                nc.tensor.matmul(
                    pq[:],
                    lhsT=wqk_sb[:, kc, t, p * 128 : (p + 1) * 128],
                    rhs=xT_sb[:, kc, tt * 512 : (tt + 1) * 512],
                    start=(kc == 0),
                    stop=(kc == DC - 1),
                )
            nc.scalar.activation(
                out=qkT_sb[:, t, p, tt * 512 : (tt + 1) * 512],
                in_=pq[:],
                func=mybir.ActivationFunctionType.Identity,
                bias=bqk_sb[:, t * 4 + p : t * 4 + p + 1],
            )

        def emit_vproj():
            # v projection: V[tok, feat] = x @ w_v. The v-bias is EXACT as a
            # host-side output constant (attention weights sum to 1, so
            # wa @ (v + bv) = wa @ v + bv): folded into b_out on the host.
            for c in range(TOK):
                pv = ps_m.tile([128, 512], mybir.dt.float32, tag="misc", name=f"pv_{c}")
                for kc in range(DC):
                    nc.tensor.matmul(
                        pv[:],
                        lhsT=xT_sb[:, kc, c * 128 : (c + 1) * 128],
                        rhs=wv_sb[:, kc, :],
                        start=(kc == 0),
                        stop=(kc == DC - 1),
                    )
                nc.vector.tensor_copy(
                    out=V_sb[:, c, :, 0:HD],
                    in_=pv[:].rearrange("p (l d) -> p l d", l=LH),
                )

        def unit(n):
            return n // 4, n % 4  # (q4, pair)

        def emit_scores_chunk(n, kc, E):
            q4, pair = unit(n)
            psc = ps_s.tile([128, 1024], mybir.dt.float32, tag="sc", name=f"sc_{n}_{kc}")
            for h01 in range(2):
                row = 64 * h01
                nc.tensor.matmul(
                    psc[:, h01 * 512 : (h01 + 1) * 512],
                    lhsT=qkT_sb[row : row + 64, 0, pair, kc * 128 : (kc + 1) * 128],
                    rhs=qkT_sb[row : row + 64, 1, pair, q4 * 512 : (q4 + 1) * 512],
                    start=True,
                    stop=True,
                )
            if kc in DVE_SET:
                # q-weights carry the 23.083 scale: one add + int16 convert
                # IS exp (Schraudolph), bitcast back to bf16 via the E view.
                nc.vector.tensor_scalar_add(
                    out=E[:, kc, :].bitcast(mybir.dt.int16),
                    in0=psc[:],
                    scalar1=EXP_B,
                )
            else:
                nc.scalar.activation(
                    out=E[:, kc, :],
                    in_=psc[:],
                    func=mybir.ActivationFunctionType.Exp,
                    scale=ACT_SCALE,
                )

        def emit_av_chunk(n, kc, E, pw, hs=(0, 1)):
            _, pair = unit(n)
            for h01 in hs:
                nc.tensor.matmul(
                    pw[h01][:],
                    lhsT=V_sb[:, kc, 2 * pair + h01, :],
                    rhs=E[:, kc, h01 * 512 : (h01 + 1) * 512],
                    start=(kc == 0),
                    stop=(kc == KC - 1),
                )

        def emit_norm_a(n, h01, pw):
            """Phase A (emitted at the end of unit n's iteration): one ACT
            copy retires the pw PSUM slot into SBUF so the next unit's AV
            chain can recycle it immediately, then ln + exp(-x) compute the
            reciprocal of the denominator row (same pinned act table as the
            softmax exp)."""
            pwS = small.tile([65, 512], mybir.dt.bfloat16, tag="pwS", name=f"pwS_{n}_{h01}")
            nc.scalar.activation(
                out=pwS[:], in_=pw[:], func=mybir.ActivationFunctionType.Copy
            )
            lg = small.tile([1, 512], mybir.dt.bfloat16, tag="lg", name=f"lg_{n}_{h01}")
            nc.scalar.activation(
                out=lg[:], in_=pwS[64:65, :], func=mybir.ActivationFunctionType.Ln
            )
            recip = small.tile([1, 512], mybir.dt.bfloat16, tag="recip", name=f"r_{n}_{h01}")
            nc.scalar.activation(
                out=recip[:],
                in_=lg[:],
                func=mybir.ActivationFunctionType.Exp,
                scale=-1.0,
            )
            return pwS, recip

        def emit_norm_b_pb(n, h01, recip):
            """Phase B part 1 (next iteration): broadcast the reciprocal
            across 64 partitions on the otherwise-idle GPSIMD engine (frees
            the PE of 32 rank-1 matmuls and their stationary-swap drains)."""
            rb = small.tile([64, 512], mybir.dt.bfloat16, tag="rb", name=f"rb_{n}_{h01}")
            nc.gpsimd.partition_broadcast(rb[:], recip[:])
            return rb

        def emit_norm_b_mul(n, h01, pwS, rb):
            """Phase B part 2 (DVE): normalized waT = wa * (1/denom); both
            operands SBUF bf16 so the DVE runs in its 2x mode."""
            q4, pair = unit(n)
            row = 64 * h01
            nc.vector.tensor_mul(
                out=waT_ring[q4][row : row + 64, pair, :],
                in0=pwS[0:64, :],
                in1=rb[:],
            )

        def emit_outproj(oq4):
            # out projection for a finished q-range; overlaps the next
            # q-range's attention stream. PSUM->SBUF moves alternate between
            # ACT and DVE to split the load; staged/stored in bf16.
            for cc in range(4):
                c = oq4 * 4 + cc
                o_sb = outp.tile([128, D], mybir.dt.bfloat16, tag="osb", name=f"o_{c}")
                for half in range(2):
                    po = ps_m.tile(
                        [128, 512], mybir.dt.float32, tag="misc", name=f"po_{c}_{half}"
                    )
                    for k4 in range(LH * HD // 128):
                        nc.tensor.matmul(
                            po[:],
                            lhsT=waT_ring[oq4][:, k4, cc * 128 : (cc + 1) * 128],
                            rhs=wout_sb[:, k4, half * 512 : (half + 1) * 512],
                            start=(k4 == 0),
                            stop=(k4 == LH * HD // 128 - 1),
                        )
                    dst = o_sb[:, half * 512 : (half + 1) * 512]
                    if half == 0:
                        nc.scalar.activation(
                            out=dst, in_=po[:], func=mybir.ActivationFunctionType.Copy
                        )
                    else:
                        nc.vector.tensor_copy(out=dst, in_=po[:])
                nc.sync.dma_start(out=out[c * 128 : (c + 1) * 128, :], in_=o_sb[:])

        # ---- prologue: all projections, then prime 3 units of scores ----
        # First 6 k-proj tiles run contraction-OUTER across idle PSUM banks
        # so each arriving xT chunk immediately feeds 6 matmuls and the PE
        # tracks the input DMA instead of idling ~1.4us per chunk.
        ko_sc = [
            ps_s.tile([128, 1024], mybir.dt.float32, tag="sc", name=f"ko_sc{i}")
            for i in range(2)
        ]
        ko_m = [
            ps_m.tile([128, 512], mybir.dt.float32, tag="misc", name=f"ko_m{i}")
            for i in range(2)
        ]
        ko_slots = [
            (ko_sc[0][:, 0:512], 0, 0), (ko_sc[0][:, 512:1024], 0, 1),
            (ko_sc[1][:, 0:512], 0, 2), (ko_sc[1][:, 512:1024], 0, 3),
            (ko_m[0][:], 1, 0), (ko_m[1][:], 1, 1),
        ]
        # DMA issues interleaved with the matmuls that consume them, so each
        # chunk's matmuls gate on just that chunk's two transfers instead of
        # a coarse all-inputs semaphore threshold.
        for kc in range(DC):
            nc.sync.dma_start(out=xT_sb[:, kc, :], in_=xT[:, kc, :])
            nc.sync.dma_start(out=wqk_sb[:, kc, 0, :], in_=wqk[:, kc, 0, :])
            for dst, p, tt in ko_slots:
                nc.tensor.matmul(
                    dst,
                    lhsT=wqk_sb[:, kc, 0, p * 128 : (p + 1) * 128],
                    rhs=xT_sb[:, kc, tt * 512 : (tt + 1) * 512],
                    start=(kc == 0),
                    stop=(kc == DC - 1),
                )
        for kc in range(DC):
            nc.sync.dma_start(out=wqk_sb[:, kc, 1, :], in_=wqk[:, kc, 1, :])
        # ko_m moves first: they free the ps_m slots the kproj-inner tiles
        # recycle, ~1us earlier than emitting them after the sc-half moves.
        for dst, p, tt in ko_slots[4:] + ko_slots[:4]:
            nc.scalar.activation(
                out=qkT_sb[:, 0, p, tt * 512 : (tt + 1) * 512],
                in_=dst,
                func=mybir.ActivationFunctionType.Identity,
                bias=bqk_sb[:, p : p + 1],
            )
        for p, tt in [(1, 2), (1, 3), (2, 0), (2, 1), (2, 2), (2, 3), (3, 0), (3, 1), (3, 2), (3, 3)]:
            emit_proj(0, p, tt)
        for p in range(4):
            emit_proj(1, p, 0)

        E_tiles = {}

        def alloc_E(n):
            E_tiles[n] = big.tile(
                [128, KC, 1024], mybir.dt.bfloat16, tag="big", name=f"E_{n}"
            )

        for n in (0, 1):
            alloc_E(n)
            for kc in range(KC):
                emit_scores_chunk(n, kc, E_tiles[n])
        for p in range(4):
            for tt in range(1, QT):
                emit_proj(1, p, tt)
        nc.sync.dma_start(out=wv_sb[:], in_=wv[:])
        emit_vproj()  # last reader of xT; E(2) recycles its big-pool slot
        nc.sync.dma_start(out=wout_sb[:], in_=wout[:])
        alloc_E(2)
        for kc in range(KC):
            emit_scores_chunk(2, kc, E_tiles[2])

        # ---- steady state: AV(n) interleaved with scores/exp(n+3) ----
        # Emission order per iteration is tuned so every cross-engine gate
        # (pw-slot recycle, psc-slot recycle, E-slot recycle) is satisfied
        # ~an iteration before the PE reaches the dependent instruction.
        LEAD = 3
        FRONT = 5
        norm_prev = None  # (pwS, recip) pairs of unit n-1
        pw_prev = None
        for n in range(15):
            m = n + LEAD
            if m <= 15:
                alloc_E(m)
            pw = [
                ps_wa.tile([65, 512], mybir.dt.float32, tag="wa", name=f"wa_{n}_{h}")
                for h in range(2)
            ]
            for kc in range(FRONT):
                emit_av_chunk(n, kc, E_tiles[n], pw)
            if m <= 15:
                for kc in range(FRONT):
                    emit_scores_chunk(m, kc, E_tiles[m])
            pbs = None
            if n >= 1:
                if (n - 1) % 4 == 0:
                    waT_ring[(n - 1) // 4] = wa_pool.tile(
                        [128, LH * HD // 128, 512], mybir.dt.bfloat16,
                        tag="waT", name=f"waT_{(n - 1) // 4}",
                    )
                pbs = [emit_norm_b_pb(n - 1, h, norm_prev[h][1]) for h in range(2)]
            for idx, j in enumerate(range(FRONT, KC - 1, 2)):
                emit_av_chunk(n, j, E_tiles[n], pw)
                emit_av_chunk(n, j + 1, E_tiles[n], pw)
                if m <= 15:
                    emit_scores_chunk(m, j, E_tiles[m])
                    emit_scores_chunk(m, j + 1, E_tiles[m])
                if idx == 1:
                    # muls late in the DVE queue (the exp chunks that gate
                    # the psc recycle come first); outproj right after the
                    # muls it depends on.
                    if pbs is not None:
                        for h01 in range(2):
                            emit_norm_b_mul(n - 1, h01, norm_prev[h01][0], pbs[h01])
                    if n % 4 == 0 and n > 0:
                        emit_outproj(n // 4 - 1)
            emit_av_chunk(n, KC - 1, E_tiles[n], pw)
            norm_prev = [emit_norm_a(n, h01, pw[h01]) for h01 in range(2)]
            if m <= 15:
                emit_scores_chunk(m, KC - 1, E_tiles[m])
            pw_prev = pw
            del E_tiles[n]
        # ---- unit 15: h0 chain -> its normalize overlaps the h1 chain, so
        # outproj(3) only waits on the short h1 normalize tail ----
        bcs14 = [emit_norm_b_pb(14, h, norm_prev[h][1]) for h in range(2)]
        pw = [
            ps_wa.tile([65, 512], mybir.dt.float32, tag="wa", name=f"wa_15_{h}")
            for h in range(2)
        ]
        waT_ring[3] = waT_ring.get(3) or wa_pool.tile(
            [128, LH * HD // 128, 512], mybir.dt.bfloat16, tag="waT", name="waT_3"
        )
        for kc in range(2):
            emit_av_chunk(15, kc, E_tiles[15], pw, hs=(0,))
        for h01 in range(2):
            emit_norm_b_mul(14, h01, norm_prev[h01][0], bcs14[h01])
        for kc in range(2, KC):
            emit_av_chunk(15, kc, E_tiles[15], pw, hs=(0,))
        n15_h0 = emit_norm_a(15, 0, pw[0])
        bc15_0 = emit_norm_b_pb(15, 0, n15_h0[1])
        emit_norm_b_mul(15, 0, n15_h0[0], bc15_0)
        for kc in range(KC):
            emit_av_chunk(15, kc, E_tiles[15], pw, hs=(1,))
        # last normalize skips the pwS staging (nothing recycles pw after
        # this) so outproj's final matmuls wait ~1us less.
        lg15 = small.tile([1, 512], mybir.dt.bfloat16, tag="lg", name="lg_15_1")
        nc.scalar.activation(
            out=lg15[:], in_=pw[1][64:65, :], func=mybir.ActivationFunctionType.Ln
        )
        r15 = small.tile([1, 512], mybir.dt.bfloat16, tag="recip", name="r_15_1")
        nc.scalar.activation(
            out=r15[:], in_=lg15[:], func=mybir.ActivationFunctionType.Exp, scale=-1.0
        )
        rb15 = small.tile([64, 512], mybir.dt.bfloat16, tag="rb", name="rb_15_1")
        nc.gpsimd.partition_broadcast(rb15[:], r15[:])
        # partial outproj chains for the first two token chunks of q4=3:
        # pairs 0-2 are normalized, so k4 0..2 accumulate during the tail
        pos_part = {}
        for cc in (0, 1):
            po = ps_m.tile([128, 512], mybir.dt.float32, tag="misc", name=f"po_p_{cc}")
            for k4 in range(3):
                nc.tensor.matmul(
                    po[:],
                    lhsT=waT_ring[3][:, k4, cc * 128 : (cc + 1) * 128],
                    rhs=wout_sb[:, k4, 0:512] if False else wout_sb[:, k4, :][:, 0:1024][:, 0:512],
                    start=(k4 == 0),
                    stop=False,
                )
            pos_part[cc] = po
        nc.vector.tensor_mul(
            out=waT_ring[3][64:128, 3, :], in0=pw[1][0:64, :], in1=rb15[:]
        )
        del E_tiles[15]
        # finish the partial chains (k4=3 needs unit 15's normalize), then
        # the remaining chunks
        for cc in (0, 1):
            c = 12 + cc
            o_sb = outp.tile([128, D], mybir.dt.bfloat16, tag="osb", name=f"o_{c}")
            po = pos_part[cc]
            nc.tensor.matmul(
                po[:],
                lhsT=waT_ring[3][:, 3, cc * 128 : (cc + 1) * 128],
                rhs=wout_sb[:, 3, 0:512],
                start=False,
                stop=True,
            )
            nc.scalar.activation(
                out=o_sb[:, 0:512], in_=po[:], func=mybir.ActivationFunctionType.Copy
            )
            po2 = ps_m.tile([128, 512], mybir.dt.float32, tag="misc", name=f"po_p2_{cc}")
            for k4 in range(4):
                nc.tensor.matmul(
                    po2[:],
                    lhsT=waT_ring[3][:, k4, cc * 128 : (cc + 1) * 128],
                    rhs=wout_sb[:, k4, 512:1024],
                    start=(k4 == 0),
                    stop=(k4 == 3),
                )
            nc.vector.tensor_copy(out=o_sb[:, 512:1024], in_=po2[:])
            nc.sync.dma_start(out=out[c * 128 : (c + 1) * 128, :], in_=o_sb[:])
        for cc in (2, 3):
            c = 12 + cc
            o_sb = outp.tile([128, D], mybir.dt.bfloat16, tag="osb", name=f"o_{c}")
            for half in range(2):
                po = ps_m.tile(
                    [128, 512], mybir.dt.float32, tag="misc", name=f"po_{c}_{half}"
                )
                for k4 in range(4):
                    nc.tensor.matmul(
                        po[:],
                        lhsT=waT_ring[3][:, k4, cc * 128 : (cc + 1) * 128],
                        rhs=wout_sb[:, k4, half * 512 : (half + 1) * 512],
                        start=(k4 == 0),
                        stop=(k4 == 3),
                    )
                dst = o_sb[:, half * 512 : (half + 1) * 512]
                if half == 0:
                    nc.scalar.activation(
                        out=dst, in_=po[:], func=mybir.ActivationFunctionType.Copy
                    )
                else:
                    nc.vector.tensor_copy(out=dst, in_=po[:])
            nc.sync.dma_start(out=out[c * 128 : (c + 1) * 128, :], in_=o_sb[:])

    nc.compile()
    return nc


def _prep_in_maps(x, w_qkv, b_qkv, w_out):
    """Host-side shard + relayout. Core c -> (batch c//2, head-group c%2)."""
    wq = w_qkv[:, :D].reshape(D, H, HD)
    wk = w_qkv[:, D : 2 * D].reshape(D, H, HD)
    wv_ = w_qkv[:, 2 * D :].reshape(D, H, HD)
    bq = b_qkv[:D].reshape(H, HD)
    bk = b_qkv[D : 2 * D].reshape(H, HD)
    bv = b_qkv[2 * D :].reshape(H, HD)
    wo = w_out.reshape(H, HD, D)

    per_group = {}
    for g in range(G):
        h0 = g * LH
        # feature order: block t=0 = k feats, t=1 = q feats (scaled by
        # QSCALE so the DVE bit-trick exp needs no multiply); within a
        # block, pair p occupies cols p*128..(p+1)*128 (first head at 0-63).
        Wqk = np.empty((D, 2, 4, 128), F32)
        Bqk = np.empty((2, 4, 128), F32)
        for p in range(LH // 2):
            ha, hb = h0 + 2 * p, h0 + 2 * p + 1
            Wqk[:, 0, p, 0:64] = wk[:, ha]
            Wqk[:, 0, p, 64:128] = wk[:, hb]
            Wqk[:, 1, p, 0:64] = wq[:, ha] * QSCALE
            Wqk[:, 1, p, 64:128] = wq[:, hb] * QSCALE
            Bqk[0, p, 0:64] = bk[ha]
            Bqk[0, p, 64:128] = bk[hb]
            Bqk[1, p, 0:64] = bq[ha] * QSCALE
            Bqk[1, p, 64:128] = bq[hb] * QSCALE
        wqk_arr = np.ascontiguousarray(
            Wqk.reshape(DC, 128, 2, 512).transpose(1, 0, 2, 3)
        ).astype(BF16)
        bqk_arr = np.ascontiguousarray(Bqk.reshape(8, 128).T)

        Wv = wv_[:, h0 : h0 + LH, :].reshape(D, LH * HD)
        wv_arr = np.ascontiguousarray(
            Wv.reshape(DC, 128, LH * HD).transpose(1, 0, 2)
        ).astype(BF16)

        Wo = wo[h0 : h0 + LH].reshape(LH * HD, D)
        wout_arr = np.ascontiguousarray(
            Wo.reshape(LH * HD // 128, 128, D).transpose(1, 0, 2)
        ).astype(BF16)
        per_group[g] = (wqk_arr, bqk_arr, wv_arr, wout_arr)

    in_maps = []
    for c in range(NCORES):
        b, g = divmod(c, G)
        wqk_arr, bqk_arr, wv_arr, wout_arr = per_group[g]
        xT_arr = np.ascontiguousarray(
            x[b].T.reshape(DC, 128, N).transpose(1, 0, 2)
        ).astype(BF16)
        in_maps.append(
            {
                "xT": xT_arr,
                "wqk": wqk_arr,
                "bqk": bqk_arr,
                "wv": wv_arr,
                "wout": wout_arr,
            }
        )
    return in_maps


def _ensure_ntff_hook():
    """Register the axon NTFF profile hook if the image's antenv lacks it.

    Mirrors trn_agent_boot.trn_boot._ntff_profile_via_ctypes: drives NRT
    profiling through the injected libaxon_pjrt.so C ABI. Without this,
    run_bass_kernel_spmd(trace=True) raises ImportError under axon.
    """
    try:
        from antenv.axon_hooks import get_axon_ntff_profile_hook  # noqa: F401

        return
    except ImportError:
        pass

    import contextlib
    import ctypes
    import types

    so_path = "/opt/axon/libaxon_pjrt.so"
    lib = ctypes.CDLL(so_path)
    if not hasattr(lib, "axon_start_nrt_profile"):
        return
    lib.axon_start_nrt_profile.argtypes = [ctypes.POINTER(ctypes.c_int64), ctypes.c_size_t]
    lib.axon_start_nrt_profile.restype = ctypes.c_int64
    lib.axon_stop_nrt_profile.argtypes = [ctypes.c_char_p]
    lib.axon_stop_nrt_profile.restype = ctypes.c_int64

    @contextlib.contextmanager
    def _hook(output_dir, device_ids):
        import jax

        jax.devices()
        if device_ids:
            ids = (ctypes.c_int64 * len(device_ids))(*device_ids)
            rc = lib.axon_start_nrt_profile(ids, len(device_ids))
        else:
            rc = lib.axon_start_nrt_profile(None, 0)
        if rc != 0:
            raise RuntimeError(f"axon_start_nrt_profile rc={rc}")
        try:
            yield
        finally:
            n = lib.axon_stop_nrt_profile(str(output_dir).encode())
            print(f"ntff profile: {n} file(s) written to {output_dir}", file=sys.stderr)

    mod = types.ModuleType("antenv.axon_hooks")
    mod.get_axon_ntff_profile_hook = lambda: _hook
    sys.modules["antenv.axon_hooks"] = mod

    # No artifact bucket in this sandbox; keep the NEFF dir local.
    from concourse import bass_utils as _bu

    _bu.upload_artifacts = lambda tmpdir: tmpdir


def kernel(x, w_qkv, b_qkv, w_out, b_out):
    x = np.asarray(x, dtype=F32)
    w_qkv = np.asarray(w_qkv, dtype=F32)
    b_qkv = np.asarray(b_qkv, dtype=F32)
    w_out = np.asarray(w_out, dtype=F32)
    b_out = np.asarray(b_out, dtype=F32)

    if "nc" not in _CACHE:
        _CACHE["nc"] = _build_nc()
    nc = _CACHE["nc"]

    in_maps = _prep_in_maps(x, w_qkv, b_qkv, w_out)
    trace = bool(int(os.environ.get("BASSMHA_TRACE", "0")))
    kwargs = {}
    if trace:
        _ensure_ntff_hook()
        tdir = os.environ.get("BASSMHA_TRACE_DIR")
        if tdir:
            os.makedirs(tdir, exist_ok=True)
            kwargs["tmpdir"] = tdir
    res = run_bass_kernel_spmd(nc, in_maps, list(range(NCORES)), trace=trace, **kwargs)
    _CACHE["last_results"] = res

    # v-bias folds exactly into an output constant: wa @ (v + bv) =
    # wa @ v + bv because the attention weights sum to 1.
    const_add = b_out + b_qkv[2 * D :].astype(F32) @ w_out
    out = np.empty((B, N, D), F32)
    for b in range(B):
        out[b] = res.results[2 * b]["out"].astype(F32)
        out[b] += res.results[2 * b + 1]["out"].astype(F32)
        out[b] += const_add
    return out
